# revision 6
# baseline (speedup 1.0000x reference)
"""Trainium2 Bass kernel for a 5x5 conv2d (NCHW, pad=2, stride=1).

Problem: X [32,32,128,128] f32, K [64,32,5,5] f32 -> out [32,64,128,128].
Sharding: data-parallel over 8 NeuronCores, 4 images per core.

Per-core mapping:
  The 4 images of the shard occupy the 4 PE row-groups (SBUF partitions
  32g..32g+31 hold image g's 32 input channels, zero-padded to 132x132 on
  the host and stored bf16). Each conv tap (dy,dx) is a K=32 x M=64
  matmul whose rhs is an access-pattern offset into the padded image.
  bf16 enables column tiling: tile_position=(32g, 64h) runs 4 row-groups
  x 2 col-groups = 8 concurrent 32x64 matmuls -> the full 128x128 array.
  Weights (replicated per row-group on the host) stay tiny in SBUF; the
  25 taps accumulate f32 in PSUM; 8 banks cover 16 output rows x 4
  images per super-round.
"""

import numpy as np

import concourse.bass as bass
import concourse.tile as tile
from concourse import bacc, mybir
from concourse.bass_utils import run_bass_kernel_spmd

N_CORES = 8
IMGS = 4          # images per core = PE row groups
C = 32            # input channels
O = 64            # output channels
H = W = 128
KH = KW = 5
PAD = 2
HP = H + 2 * PAD  # 132 padded rows
WP = W + 2 * PAD  # 132 padded row length
TAPS = KH * KW    # 25
RT = 4            # output rows per psum half-bank (RT*W = 512 = bank)
SR_ROWS = 16      # output rows per super-round (2 banks x 2 halves)
N_SR = H // SR_ROWS

F32 = mybir.dt.float32
BF16 = mybir.dt.bfloat16


OUT_BF16 = True  # store output bf16, host upconverts (halves out DMA)


def _build_nc(reps=1):
    out_dt = BF16 if OUT_BF16 else F32
    nc = bacc.Bacc("TRN2", target_bir_lowering=False, debug=False)
    XP = nc.dram_tensor("XP", [IMGS * C, HP, WP], BF16, kind="ExternalInput").ap()
    KT = nc.dram_tensor("KT", [IMGS * C, TAPS, O], BF16, kind="ExternalInput").ap()
    out = nc.dram_tensor("out", [IMGS, O, H, W], out_dt, kind="ExternalOutput").ap()

    with tile.TileContext(nc) as tc:
        with (
            tc.tile_pool(name="wpool", bufs=1) as wpool,
            tc.tile_pool(name="xpool", bufs=2) as xpool,
            tc.tile_pool(name="opool", bufs=8) as opool,
            tc.tile_pool(name="ppool", bufs=8, space="PSUM") as ppool,
        ):
            # Weights: partition 32g+c holds K[o, c, tap] for image-group g
            # (pre-replicated on the host so every PE row-group loads its
            # stationary operand from its own partitions).
            wt = wpool.tile([IMGS * C, TAPS, O], BF16)
            nc.sync.dma_start(wt[:, :, :], KT)

            def body():
                xt = xpool.tile([IMGS * C, HP, WP], BF16)
                for g in range(IMGS):
                    nc.sync.dma_start(
                        xt[C * g : C * (g + 1), :, :], XP[C * g : C * (g + 1), :, :]
                    )
                for sr in range(N_SR):
                    y0 = SR_ROWS * sr
                    pss = [
                        ppool.tile(
                            [2 * O, RT, W], F32, name=f"ps_s{sr}_i{i}", tag="ps"
                        )
                        for i in range(2 * IMGS)
                    ]
                    for t in range(TAPS):
                        dy, dx = t // KW, t % KW
                        first = t == 0
                        last = t == TAPS - 1
                        for h in range(2):
                            for b in range(2):
                                for g in range(IMGS):
                                    lhsT = wt[C * g : C * (g + 1), t, :]
                                    # output rows y0+8b+4h .. +3; padded input
                                    # row index = output row + dy
                                    r0 = y0 + 8 * b + 4 * h + dy
                                    nc.tensor.matmul(
                                        pss[2 * g + b][
                                            O * h : O * (h + 1), :, :
                                        ],
                                        lhsT,
                                        xt[
                                            C * g : C * (g + 1),
                                            r0 : r0 + RT,
                                            dx : dx + W,
                                        ],
                                        start=first,
                                        stop=last,
                                        tile_position=(C * g, O * h),
                                    )
                    for g in range(IMGS):
                        for b in range(2):
                            ob = opool.tile([2 * O, RT, W], out_dt)
                            nc.any.tensor_copy(ob[:, :, :], pss[2 * g + b][:, :, :])
                            yb = y0 + 8 * b
                            nc.sync.dma_start(
                                out[g, :, yb : yb + RT, :], ob[0:O, :, :]
                            )
                            nc.sync.dma_start(
                                out[g, :, yb + RT : yb + 2 * RT, :],
                                ob[O : 2 * O, :, :],
                            )

            if reps > 1:
                with tc.For_i(0, reps, 1):
                    body()
            else:
                body()
    nc.compile()
    return nc


_CACHE = {}


def _get_nc(reps=1):
    if reps not in _CACHE:
        _CACHE[reps] = _build_nc(reps)
    return _CACHE[reps]


def _prep_inputs(X, K):
    """Host-side: pad + cast X, replicate + cast K. Returns per-core in_maps."""
    import ml_dtypes

    bf16 = ml_dtypes.bfloat16
    X = np.asarray(X, dtype=np.float32)
    K = np.asarray(K, dtype=np.float32)
    n = X.shape[0]
    per = n // N_CORES
    XPad = np.zeros((n, C, HP, WP), dtype=bf16)
    XPad[:, :, PAD : PAD + H, PAD : PAD + W] = X.astype(bf16)
    # KT[32g+c, t, o] = K[o, c, t]
    KT = np.tile(
        np.ascontiguousarray(K.transpose(1, 2, 3, 0)).reshape(C, TAPS, O),
        (IMGS, 1, 1),
    ).astype(bf16)
    return [
        {
            "XP": np.ascontiguousarray(
                XPad[per * i : per * (i + 1)].reshape(per * C, HP, WP)
            ),
            "KT": KT,
        }
        for i in range(N_CORES)
    ]


def make_in_maps(X, K):
    return _prep_inputs(X, K)


def kernel(X, K):
    nc = _get_nc()
    in_maps = _prep_inputs(X, K)
    res = run_bass_kernel_spmd(nc, in_maps, list(range(N_CORES))).results
    return np.concatenate(
        [np.asarray(res[i]["out"], dtype=np.float32) for i in range(N_CORES)],
        axis=0,
    )


# revision 7
# speedup vs baseline: 1.2660x; 1.2660x over previous
"""Trainium2 Bass kernel for a 5x5 conv2d (NCHW, pad=2, stride=1).

Problem: X [32,32,128,128] f32, K [64,32,5,5] f32 -> out [32,64,128,128].
Sharding: data-parallel over 8 NeuronCores, 4 images per core.

Per-core mapping:
  The 4 images of the shard occupy the 4 PE row-groups (SBUF partitions
  32g..32g+31 hold image g's 32 input channels, zero-padded to 132x132 on
  the host and stored bf16). Each conv tap (dy,dx) is a K=32 x M=64
  matmul whose rhs is an access-pattern offset into the padded image.
  bf16 enables column tiling: tile_position=(32g, 64h) runs 4 row-groups
  x 2 col-groups = 8 concurrent 32x64 matmuls -> the full 128x128 array.
  Weights (replicated per row-group on the host) stay tiny in SBUF; the
  25 taps accumulate f32 in PSUM; 8 banks cover 16 output rows x 4
  images per super-round.
"""

import numpy as np

import concourse.bass as bass
import concourse.tile as tile
from concourse import bacc, mybir
from concourse.bass_utils import run_bass_kernel_spmd

N_CORES = 8
IMGS = 4          # images per core = PE row groups
C = 32            # input channels
O = 64            # output channels
H = W = 128
KH = KW = 5
PAD = 2
HP = H + 2 * PAD  # 132 padded rows
WP = W + 2 * PAD  # 132 padded row length
TAPS = KH * KW    # 25
RT = 4            # output rows per psum half-bank (RT*W = 512 = bank)
SR_ROWS = 8       # output rows per round (1 bank x 2 halves per image)
NB = SR_ROWS // 8  # banks per image per round
N_SR = H // SR_ROWS

F32 = mybir.dt.float32
BF16 = mybir.dt.bfloat16


OUT_BF16 = True  # store output bf16, host upconverts (halves out DMA)


def _build_nc(reps=1):
    out_dt = BF16 if OUT_BF16 else F32
    nc = bacc.Bacc("TRN2", target_bir_lowering=False, debug=False)
    XP = nc.dram_tensor("XP", [IMGS * C, HP, WP], BF16, kind="ExternalInput").ap()
    KT = nc.dram_tensor("KT", [IMGS * C, TAPS, O], BF16, kind="ExternalInput").ap()
    out = nc.dram_tensor("out", [IMGS, O, H, W], out_dt, kind="ExternalOutput").ap()

    with tile.TileContext(nc) as tc:
        with (
            tc.tile_pool(name="wpool", bufs=1) as wpool,
            tc.tile_pool(name="xpool", bufs=2) as xpool,
            tc.tile_pool(name="opool", bufs=8) as opool,
            tc.tile_pool(name="ppool", bufs=8, space="PSUM") as ppool,
        ):
            # Weights: partition 32g+c holds K[o, c, tap] for image-group g
            # (pre-replicated on the host so every PE row-group loads its
            # stationary operand from its own partitions).
            wt = wpool.tile([IMGS * C, TAPS, O], BF16)
            nc.sync.dma_start(wt[:, :, :], KT)

            def body():
                xt = xpool.tile([IMGS * C, HP, WP], BF16)
                for g in range(IMGS):
                    nc.sync.dma_start(
                        xt[C * g : C * (g + 1), :, :], XP[C * g : C * (g + 1), :, :]
                    )
                for sr in range(N_SR):
                    y0 = SR_ROWS * sr
                    pss = [
                        ppool.tile(
                            [2 * O, RT, W], F32, name=f"ps_s{sr}_i{i}", tag="ps"
                        )
                        for i in range(NB * IMGS)
                    ]
                    for t in range(TAPS):
                        dy, dx = t // KW, t % KW
                        first = t == 0
                        last = t == TAPS - 1
                        for h in range(2):
                            for b in range(NB):
                                for g in range(IMGS):
                                    lhsT = wt[C * g : C * (g + 1), t, :]
                                    # output rows y0+8b+4h .. +3; padded input
                                    # row index = output row + dy
                                    r0 = y0 + 8 * b + 4 * h + dy
                                    nc.tensor.matmul(
                                        pss[NB * g + b][
                                            O * h : O * (h + 1), :, :
                                        ],
                                        lhsT,
                                        xt[
                                            C * g : C * (g + 1),
                                            r0 : r0 + RT,
                                            dx : dx + W,
                                        ],
                                        start=first,
                                        stop=last,
                                        tile_position=(C * g, O * h),
                                    )
                    for g in range(IMGS):
                        for b in range(NB):
                            ob = opool.tile([2 * O, RT, W], out_dt)
                            nc.any.tensor_copy(ob[:, :, :], pss[NB * g + b][:, :, :])
                            yb = y0 + 8 * b
                            nc.sync.dma_start(
                                out[g, :, yb : yb + RT, :], ob[0:O, :, :]
                            )
                            nc.sync.dma_start(
                                out[g, :, yb + RT : yb + 2 * RT, :],
                                ob[O : 2 * O, :, :],
                            )

            if reps > 1:
                with tc.For_i(0, reps, 1):
                    body()
            else:
                body()
    nc.compile()
    return nc


_CACHE = {}


def _get_nc(reps=1):
    if reps not in _CACHE:
        _CACHE[reps] = _build_nc(reps)
    return _CACHE[reps]


def _prep_inputs(X, K):
    """Host-side: pad + cast X, replicate + cast K. Returns per-core in_maps."""
    import ml_dtypes

    bf16 = ml_dtypes.bfloat16
    X = np.asarray(X, dtype=np.float32)
    K = np.asarray(K, dtype=np.float32)
    n = X.shape[0]
    per = n // N_CORES
    XPad = np.zeros((n, C, HP, WP), dtype=bf16)
    XPad[:, :, PAD : PAD + H, PAD : PAD + W] = X.astype(bf16)
    # KT[32g+c, t, o] = K[o, c, t]
    KT = np.tile(
        np.ascontiguousarray(K.transpose(1, 2, 3, 0)).reshape(C, TAPS, O),
        (IMGS, 1, 1),
    ).astype(bf16)
    return [
        {
            "XP": np.ascontiguousarray(
                XPad[per * i : per * (i + 1)].reshape(per * C, HP, WP)
            ),
            "KT": KT,
        }
        for i in range(N_CORES)
    ]


def make_in_maps(X, K):
    return _prep_inputs(X, K)


def kernel(X, K):
    nc = _get_nc()
    in_maps = _prep_inputs(X, K)
    res = run_bass_kernel_spmd(nc, in_maps, list(range(N_CORES))).results
    return np.concatenate(
        [np.asarray(res[i]["out"], dtype=np.float32) for i in range(N_CORES)],
        axis=0,
    )


# revision 10
# speedup vs baseline: 1.2689x; 1.0023x over previous
"""Trainium2 Bass kernel for a 5x5 conv2d (NCHW, pad=2, stride=1).

Problem: X [32,32,128,128] f32, K [64,32,5,5] f32 -> out [32,64,128,128].
Sharding: data-parallel over 8 NeuronCores, 4 images per core.

Per-core mapping:
  The 4 images of the shard occupy the 4 PE row-groups (SBUF partitions
  32g..32g+31 hold image g's 32 input channels, zero-padded to 132x132 on
  the host and stored bf16). Each conv tap (dy,dx) is a K=32 x M=64
  matmul whose rhs is an access-pattern offset into the padded image.
  bf16 enables column tiling: tile_position=(32g, 64h) runs 4 row-groups
  x 2 col-groups = 8 concurrent 32x64 matmuls -> the full 128x128 array.
  Weights (replicated per row-group on the host) stay tiny in SBUF; the
  25 taps accumulate f32 in PSUM; 8 banks cover 16 output rows x 4
  images per super-round.
"""

import numpy as np

import concourse.bass as bass
import concourse.tile as tile
from concourse import bacc, mybir
from concourse.bass_utils import run_bass_kernel_spmd

N_CORES = 8
IMGS = 4          # images per core = PE row groups
C = 32            # input channels
O = 64            # output channels
H = W = 128
KH = KW = 5
PAD = 2
HP = H + 2 * PAD  # 132 padded rows
WP = W + 2 * PAD  # 132 padded row length
TAPS = KH * KW    # 25
RT = 4            # output rows per psum half-bank (RT*W = 512 = bank)
SR_ROWS = 8       # output rows per round (1 bank x 2 halves per image)
NB = SR_ROWS // 8  # banks per image per round
N_SR = H // SR_ROWS

F32 = mybir.dt.float32
BF16 = mybir.dt.bfloat16


OUT_BF16 = True  # store output bf16, host upconverts (halves out DMA)


def _build_nc(reps=1):
    out_dt = BF16 if OUT_BF16 else F32
    nc = bacc.Bacc("TRN2", target_bir_lowering=False, debug=False)
    XP = nc.dram_tensor("XP", [IMGS * C, HP, WP], BF16, kind="ExternalInput").ap()
    KT = nc.dram_tensor("KT", [IMGS * C, TAPS, O], BF16, kind="ExternalInput").ap()
    out = nc.dram_tensor("out", [IMGS, O, H, W], out_dt, kind="ExternalOutput").ap()

    with tile.TileContext(nc) as tc:
        with (
            tc.tile_pool(name="wpool", bufs=1) as wpool,
            tc.tile_pool(name="xpool", bufs=2) as xpool,
            tc.tile_pool(name="opool", bufs=16) as opool,
            tc.tile_pool(name="ppool", bufs=8, space="PSUM") as ppool,
        ):
            # Weights: partition 32g+c holds K[o, c, tap] for image-group g
            # (pre-replicated on the host so every PE row-group loads its
            # stationary operand from its own partitions).
            wt = wpool.tile([IMGS * C, TAPS, O], BF16)
            nc.sync.dma_start(wt[:, :, :], KT)

            def body():
                xt = xpool.tile([IMGS * C, HP, WP], BF16)
                for g in range(IMGS):
                    nc.sync.dma_start(
                        xt[C * g : C * (g + 1), :, :], XP[C * g : C * (g + 1), :, :]
                    )
                for sr in range(N_SR):
                    y0 = SR_ROWS * sr
                    pss = [
                        ppool.tile(
                            [2 * O, RT, W], F32, name=f"ps_s{sr}_i{i}", tag="ps"
                        )
                        for i in range(NB * IMGS)
                    ]
                    for t in range(TAPS):
                        dy, dx = t // KW, t % KW
                        first = t == 0
                        last = t == TAPS - 1
                        for h in range(2):
                            for b in range(NB):
                                for g in range(IMGS):
                                    lhsT = wt[C * g : C * (g + 1), t, :]
                                    # output rows y0+8b+4h .. +3; padded input
                                    # row index = output row + dy
                                    r0 = y0 + 8 * b + 4 * h + dy
                                    nc.tensor.matmul(
                                        pss[NB * g + b][
                                            O * h : O * (h + 1), :, :
                                        ],
                                        lhsT,
                                        xt[
                                            C * g : C * (g + 1),
                                            r0 : r0 + RT,
                                            dx : dx + W,
                                        ],
                                        start=first,
                                        stop=last,
                                        tile_position=(C * g, O * h),
                                    )
                    for g in range(IMGS):
                        for b in range(NB):
                            ob = opool.tile([2 * O, RT, W], out_dt)
                            nc.any.tensor_copy(ob[:, :, :], pss[NB * g + b][:, :, :])
                            yb = y0 + 8 * b
                            nc.sync.dma_start(
                                out[g, :, yb : yb + RT, :], ob[0:O, :, :]
                            )
                            nc.sync.dma_start(
                                out[g, :, yb + RT : yb + 2 * RT, :],
                                ob[O : 2 * O, :, :],
                            )

            if reps > 1:
                with tc.For_i(0, reps, 1):
                    body()
            else:
                body()
    nc.compile()
    return nc


_CACHE = {}


def _get_nc(reps=1):
    if reps not in _CACHE:
        _CACHE[reps] = _build_nc(reps)
    return _CACHE[reps]


def _prep_inputs(X, K):
    """Host-side: pad + cast X, replicate + cast K. Returns per-core in_maps."""
    import ml_dtypes

    bf16 = ml_dtypes.bfloat16
    X = np.asarray(X, dtype=np.float32)
    K = np.asarray(K, dtype=np.float32)
    n = X.shape[0]
    per = n // N_CORES
    XPad = np.zeros((n, C, HP, WP), dtype=bf16)
    XPad[:, :, PAD : PAD + H, PAD : PAD + W] = X.astype(bf16)
    # KT[32g+c, t, o] = K[o, c, t]
    KT = np.tile(
        np.ascontiguousarray(K.transpose(1, 2, 3, 0)).reshape(C, TAPS, O),
        (IMGS, 1, 1),
    ).astype(bf16)
    return [
        {
            "XP": np.ascontiguousarray(
                XPad[per * i : per * (i + 1)].reshape(per * C, HP, WP)
            ),
            "KT": KT,
        }
        for i in range(N_CORES)
    ]


def make_in_maps(X, K):
    return _prep_inputs(X, K)


def kernel(X, K):
    nc = _get_nc()
    in_maps = _prep_inputs(X, K)
    res = run_bass_kernel_spmd(nc, in_maps, list(range(N_CORES))).results
    return np.concatenate(
        [np.asarray(res[i]["out"], dtype=np.float32) for i in range(N_CORES)],
        axis=0,
    )
